# revision 27
# baseline (speedup 1.0000x reference)
"""Trainium2 Bass kernel for the Adapter + FFT-low-pass nn.Module.

Math: the fft2 -> center-square mask -> ifft2 -> real -> abs block is a
linear operator separable over the two 64-sized spatial axes:
    Y = | A X A^T - B X B^T |   per (batch, channel) 64x64 image,
where C = IDFT @ diag(mask_unshifted) @ DFT (complex 64x64), A = Re C,
B = Im C.  Everything becomes TensorEngine matmuls.

Per core (2 of 16 batch images, 8192 tokens, pure data parallel):
  P1: h = gelu(x @ W1^T + b1)            tiles [tok(h-major), 192]
  P2: UA = (A over W) h ; UB = (B over W) h  (blockdiag stationary)
      scatter (b,h,w) -> (b,w,h) via internal-DRAM roundtrip
  P3: y = | (A over H) UA - (B over H) UB |  -> y_dr [128, 2, tok']
      (fp8 DoubleRow K-tile layout: block0 = d 0:128, block1 = d 128:192)
  P4: out[c, tok'] = W2 @ y  via fp8 DoubleRow matmuls, W2 stationary,
      K=192 in one matmul, N=512 tokens per matmul.
Software-pipelined: P3/P4 of image b-1 interleave with P1/P2 of image b
at tile-pair granularity to keep Tensor, Vector and Scalar all busy and
the PE HAM clock warm.  Skip connection + b2 are added host-side.

Output leaves in (c, b, w, h) order; host transposes back.
"""

import sys
import types

sys.path.insert(0, "/opt/trn_rl_repo")

import numpy as np

# ---------------------------------------------------------------------------
# optional NTFF profiling hook (used when trace=True; harmless otherwise)
if "antenv.axon_hooks" not in sys.modules:
    _hookmod = types.ModuleType("antenv.axon_hooks")
    _store = {}
    _hookmod.set_axon_ntff_profile_hook = lambda h: _store.__setitem__("v", h)
    _hookmod.get_axon_ntff_profile_hook = lambda: _store.get("v")
    sys.modules["antenv.axon_hooks"] = _hookmod
    try:
        from trn_agent_boot.trn_boot import _ntff_profile_via_ctypes

        _hookmod.set_axon_ntff_profile_hook(
            _ntff_profile_via_ctypes("/opt/axon/libaxon_pjrt.so")
        )
    except Exception:
        pass

import bass_rust
import concourse.bass as bass
import concourse.bacc as bacc
import concourse.mybir as mybir
import concourse.tile as tile
from concourse.bass_utils import run_bass_kernel_spmd
from concourse.tile_rust import add_dep_helper
from ml_dtypes import bfloat16, float8_e4m3

# ---------------------------------------------------------------------------
N_CORES = 8
B, H, W, C = 16, 64, 64, 768
DH = 192
B_LOC = B // N_CORES          # 2 batch images per core
TOK = B_LOC * H * W           # 8192 tokens per core
NT_B = H * W // 128           # 32 token tiles per batch image
NP_B = NT_B // 2              # 16 tile-pairs per image
KC = C // 128                 # 6 contraction chunks over channels
NG_B = H * W // 512           # 8 token groups (512) per image for stage3
F32 = mybir.dt.float32
BF16 = mybir.dt.bfloat16
FP8 = mybir.dt.float8e4
GELU = mybir.ActivationFunctionType.Gelu
ABS = mybir.ActivationFunctionType.Abs
COPY = mybir.ActivationFunctionType.Copy
DR = mybir.MatmulPerfMode.DoubleRow
DRSW = mybir.MatmulPerfMode.DoubleRowSwInterleave

DELAY_PAIRS = 4               # p3p4(b-1) trails p1p2(b) by this many pairs
DRAIN_PAT = "VSVSVS"          # p4 psum-drain engine per cc chunk


def _fft_mats():
    """A = Re(C), B = Im(C) with C = ifft(diag(m) fft(.)), N=64, RATE=.25."""
    n = 64
    line = int((n * n * 0.25) ** 0.5 // 2)
    m_shift = np.zeros(n, dtype=np.float64)
    m_shift[n // 2 - line : n // 2 + line] = 1.0
    m = np.fft.ifftshift(m_shift)
    F = np.fft.fft(np.eye(n), axis=0)
    Cm = (np.conj(F) / n) @ np.diag(m) @ F
    return np.real(Cm), np.imag(Cm)


def _blockdiag2(M):
    Z = np.zeros((128, 128), dtype=np.float64)
    Z[:64, :64] = M
    Z[64:, 64:] = M
    return Z


def build_bass():
    """Single-core Bass program, SPMD-replicated across the 8 cores."""
    nc = bacc.Bacc("TRN2", target_bir_lowering=False, debug=False,
                   num_devices=N_CORES)

    xT2 = nc.declare_dram_parameter("xT2", [128, 3, 2 * TOK], FP8,
                                    isOutput=False)
    w1p = nc.declare_dram_parameter("w1p", [128, 3, 2, DH], FP8,
                                    isOutput=False)
    w2i = nc.declare_dram_parameter("w2i", [128, KC, 2, 128], FP8,
                                    isOutput=False)
    ablk2a = nc.declare_dram_parameter("ablk2a", [128, 128], BF16,
                                       isOutput=False)
    bblk2a = nc.declare_dram_parameter("bblk2a", [128, 128], BF16,
                                       isOutput=False)
    ablk = nc.declare_dram_parameter("ablk", [128, 128], BF16, isOutput=False)
    nbblk = nc.declare_dram_parameter("nbblk", [128, 128], BF16, isOutput=False)
    onesb1 = nc.declare_dram_parameter("onesb1", [128, 128 + 2 * DH], BF16,
                                       isOutput=False)
    out = nc.declare_dram_parameter("out", [C, TOK], FP8, isOutput=True)

    # internal DRAM for the (b,h,w)->(b,w,h) scatter; [A-d | B-d] per token
    uab = nc.dram_tensor("uab", [B_LOC, H * W, 2 * DH], FP8)
    # scatter view: [b, h2, w, t, d] with token' = w*64 + (t*2 + h2)
    uab_sc = uab.rearrange("b (w t h2) d -> b h2 w t d", h2=2, t=NT_B)
    # 2b load view: [b, t4-group, p, i, d] with token' = t4*512 + i*128 + p
    uab_ld = uab.rearrange("b (t4 i p) d -> b t4 p i d", i=4, p=128)

    with tile.TileContext(nc) as tc:
        with (
            tc.tile_pool(name="const", bufs=1) as constp,
            tc.tile_pool(name="xt", bufs=4) as xtp,
            tc.tile_pool(name="h1", bufs=2) as h1p,
            tc.tile_pool(name="sa", bufs=2) as sap,
            tc.tile_pool(name="ub", bufs=5) as ubp,
            tc.tile_pool(name="yd", bufs=2) as ydp,
            tc.tile_pool(name="osb", bufs=2) as osbp,
            tc.tile_pool(name="ps1", bufs=2, space="PSUM") as ps1p,
            tc.tile_pool(name="ps2", bufs=2, space="PSUM") as ps2p,
            tc.tile_pool(name="ps3", bufs=2, space="PSUM") as ps3p,
            tc.tile_pool(name="ps4", bufs=2, space="PSUM") as ps4p,
        ):
            state = {}

            def load_xchunk(b, c):
                if ("xg", b, c) in state or c >= 8:
                    return
                t_ = xtp.tile([128, 3, 1024], FP8, tag="xg")
                nc.sync.dma_start(
                    t_[:], xT2[:, :, b * 8192 + c * 1024 :
                               b * 8192 + (c + 1) * 1024])
                state[("xg", b, c)] = t_

            # ---- first x chunks before the other constants: the first
            # stage1 matmul needs xg(0,0)+w1p+onesb1 only.
            load_xchunk(0, 0)
            w1p_sb = constp.tile([128, 3, 2, DH], FP8, tag="w1p")
            nc.sync.dma_start(w1p_sb[:], w1p[:])
            onesb1_sb = constp.tile([128, 128 + 2 * DH], BF16, tag="onesb1")
            nc.sync.dma_start(onesb1_sb[:], onesb1[:])
            load_xchunk(0, 1)
            ablk2a_sb = constp.tile([128, 128], BF16, tag="ablk2a")
            nc.gpsimd.dma_start(ablk2a_sb[:], ablk2a[:])
            bblk2a_sb = constp.tile([128, 128], BF16, tag="bblk2a")
            nc.gpsimd.dma_start(bblk2a_sb[:], bblk2a[:])
            ablk_sb = constp.tile([128, 128], BF16, tag="ablk")
            nc.gpsimd.dma_start(ablk_sb[:], ablk[:])
            nbblk_sb = constp.tile([128, 128], BF16, tag="nbblk")
            nc.gpsimd.dma_start(nbblk_sb[:], nbblk[:])
            w2i_sb = constp.tile([128, KC, 2, 128], FP8, tag="w2i")
            nc.gpsimd.dma_start(w2i_sb[:], w2i[:])
            ones_sb = onesb1_sb[:, 0:128]
            b1row2_sb = onesb1_sb[:, 128 : 128 + 2 * DH]

            # pre-zero PSUM banks used by p3: the batched abs reads a
            # never-written quadrant; keep it finite.  (ps1's zeroing is
            # deferred to after the head so it doesn't gate the first matmul.)
            for _ in range(2):
                z = ps3p.tile([128, 2, 2, 128], F32, tag="ps3")
                nc.vector.memset(z[:], 0.0)

            scat_dmas = [[], []]
            uab_fence = [None, None]

            def p12_pair(b, u):
                """stage1 + 2a for tiles 2u, 2u+1 of image b."""
                c = u // 2
                if u % 2 == 0:
                    load_xchunk(b, c)
                    load_xchunk(b, c + 1)
                    load_xchunk(b, c + 2)
                    if c >= 5 and b == 0:
                        load_xchunk(1, c - 5)
                if u == 0:
                    h1 = h1p.tile([128, NT_B, DH], FP8, tag="h1")
                    state[("h1", b)] = h1
                    sa = sap.tile([128, NT_B, 2 * DH], FP8, tag="sa")
                    state[("sa", b)] = sa
                h1 = state[("h1", b)]
                sa = state[("sa", b)]

                # --- stage1: bias first (sets has_written), then accumulate
                xg = state[("xg", b, u // 2)]
                hps = ps1p.tile([128, 2, DH], F32, tag="ps1")
                nc.tensor.matmul(hps[:], ones_sb, b1row2_sb,
                                 start=True, stop=False, skip_group_check=True)
                for i in range(2):
                    t = 2 * u + i
                    off = (t % 4) * 256
                    for j in range(3):
                        nc.tensor.matmul(
                            hps[:, i, :],
                            xg[:, j, off : off + 256].rearrange(
                                "p (i t) -> p i t", i=2),
                            w1p_sb[:, j, :, :], start=False,
                            stop=(i == 1 and j == 2),
                            skip_group_check=True, perf_mode=DRSW)
                nc.scalar.activation(h1[:, 2 * u : 2 * u + 2, :], hps[:], GELU)
                # --- 2a + sa copy
                for i in range(2):
                    t = 2 * u + i
                    aps = ps2p.tile([128, 2, DH], F32, tag="ps2")
                    nc.tensor.matmul(aps[:, 0, :], ablk2a_sb[:], h1[:, t, :],
                                     start=True, stop=True)
                    nc.tensor.matmul(aps[:, 1, :], bblk2a_sb[:],
                                     h1[:, t, :], start=True, stop=True)
                    if t % 8 >= 6:
                        nc.scalar.activation(sa[:, t, :], aps[:], COPY)
                    else:
                        nc.vector.tensor_copy(sa[:, t, :], aps[:])
                # --- scatter every 2 pairs (4 tiles); alternate DMA queues
                if u % 2 == 1:
                    t4 = u // 2
                    for h2 in range(2):
                        eng = nc.gpsimd if h2 == 0 else nc.sync
                        s = eng.dma_start(
                            uab_sc[b, h2, :, 4 * t4 : 4 * t4 + 4, :],
                            sa[h2 * 64 : (h2 + 1) * 64,
                               4 * t4 : 4 * t4 + 4, :])
                        scat_dmas[b].append(s.ins)
                if u == NP_B - 1:
                    fence = nc.sync.nop(hint=f"uab_fence_{b}", nofuse=True)
                    for s in scat_dmas[b]:
                        add_dep_helper(fence.ins, s,
                                       reason="uab fence on scatter writes")
                    uab_fence[b] = fence.ins
                    load_ub(b, 0)
                    load_ub(b, 1)
                    load_ub(b, 2)

            def load_ub(b, t4):
                if ("ubg", b, t4) in state or t4 >= NT_B // 4:
                    return
                ub = ubp.tile([128, 4, 2 * DH], FP8, tag="ub")
                ud = nc.gpsimd.dma_start(ub[:], uab_ld[b, t4, :, :, :])
                add_dep_helper(ud.ins, uab_fence[b],
                               reason="uab RAW: 2b read after 2a scatters")
                state[("ubg", b, t4)] = ub

            def p3_pair(b, u, pools):
                """2b for tiles 2u, 2u+1: y = |A.UA - B.UB| in DR layout."""
                if u == 0:
                    yd = ydp.tile([128, 2, H * W], FP8, tag="yd")
                    state[("yd", b)] = yd
                yd = state[("yd", b)]
                t4 = u // 2
                load_ub(b, t4)
                load_ub(b, t4 + 1)
                load_ub(b, t4 + 2)
                ub = state[("ubg", b, t4)]
                # psum layout [kt, i, tok]: kt-major so the batched abs AP
                # traversal matches yd's [kt, tok] order.
                pool, tg = pools[u % len(pools)]
                yps = pool.tile([128, 2, 2, 128], F32, tag=tg)
                for i in range(2):
                    j = (2 * u + i) % 4          # position within the ub group
                    nc.tensor.matmul(yps[:, 0, i, :], ub[:, j, 0:128],
                                     ablk_sb[:], start=True, stop=False,
                                     skip_group_check=True)
                    nc.tensor.matmul(yps[:, 0, i, :], ub[:, j, DH : DH + 128],
                                     nbblk_sb[:], start=False, stop=True,
                                     skip_group_check=True)
                    nc.tensor.matmul(yps[0:64, 1, i, :], ub[:, j, 128:DH],
                                     ablk_sb[:], start=True, stop=False,
                                     skip_group_check=True)
                    nc.tensor.matmul(yps[0:64, 1, i, :],
                                     ub[:, j, DH + 128 : 2 * DH],
                                     nbblk_sb[:], start=False, stop=True,
                                     skip_group_check=True)
                nc.scalar.activation(
                    yd[:, :, 2 * u * 128 : (2 * u + 2) * 128], yps[:], ABS)

            def p4_group(b, g, pools, pat=DRAIN_PAT):
                """stage3 for token group g: out[c, tok'] via fp8 DoubleRow."""
                if g == 0:
                    osb = osbp.tile([128, KC, H * W], FP8, tag="osb")
                    state[("osb", b)] = osb
                yd = state[("yd", b)]
                osb = state[("osb", b)]
                for cc in range(KC):
                    pool, tg = pools[cc % len(pools)]
                    ops = pool.tile([128, 512], F32, tag=tg)
                    drain_eng = pat[cc]
                    nc.tensor.matmul(
                        ops[:], w2i_sb[:, cc, :, :],
                        yd[:, :, g * 512 : (g + 1) * 512],
                        start=True, stop=True, perf_mode=DRSW)
                    if drain_eng == "V":
                        nc.vector.tensor_copy(
                            osb[:, cc, g * 512 : (g + 1) * 512], ops[:])
                    else:
                        nc.scalar.activation(
                            osb[:, cc, g * 512 : (g + 1) * 512], ops[:], COPY)
                # stream the finished token-group out immediately so the last
                # image's output transfer isn't exposed at the end.
                for cc in range(KC):
                    eng = nc.sync if cc % 2 == 0 else nc.gpsimd
                    eng.dma_start(
                        out[cc * 128 : (cc + 1) * 128,
                            b * H * W + g * 512 : b * H * W + (g + 1) * 512],
                        osb[:, cc, g * 512 : (g + 1) * 512])

            MID_P3 = [(ps3p, "ps3")]
            MID_P4 = [(ps4p, "ps4")]
            TAIL_P3 = [(ps3p, "ps3"), (ps1p, "ps1")]
            TAIL_P4 = [(ps4p, "ps4"), (ps2p, "ps2")]

            def p34_slot(b, v, tail):
                p3_pair(b, v, TAIL_P3 if tail else MID_P3)
                if v % 2 == 1:
                    p4_group(b, v // 2, TAIL_P4 if tail else MID_P4,
                             "VSVVSV" if tail else DRAIN_PAT)

            # ---- software-pipelined emission
            for u in range(NP_B):
                p12_pair(0, u)
            for _ in range(2):
                z = ps1p.tile([128, 2, 2, 128], F32, tag="ps1")
                nc.vector.memset(z[:], 0.0)
            for u in range(NP_B):
                p12_pair(1, u)
                v = u - DELAY_PAIRS
                if v >= 0:
                    p34_slot(0, v, tail=False)
            for v in range(NP_B - DELAY_PAIRS, NP_B):
                p34_slot(0, v, tail=True)
            for v in range(NP_B):
                p34_slot(1, v, tail=True)
    return nc


_NC_CACHE = {}


def _get_nc():
    if "nc" not in _NC_CACHE:
        nc = build_bass()
        nc.compile()
        _NC_CACHE["nc"] = nc
    return _NC_CACHE["nc"]


def make_in_maps(x, W1, b1, W2, b2):
    A, Bm = _fft_mats()
    # stage1 weights as fp8 DoubleRow pairs: w1p[p, j, i, d] = W1[d, (2j+i)*128+p]
    w1p = np.ascontiguousarray(
        W1.T.reshape(3, 2, 128, DH).transpose(2, 0, 1, 3)).astype(float8_e4m3)
    # 2a stationary filters, row-flipped to undo stage1's SwInterleave
    # token reversal (h1 partition p holds token 127-p).
    ablk2a = _blockdiag2(A.T)[::-1, :].astype(bfloat16)
    bblk2a = _blockdiag2(Bm.T)[::-1, :].astype(bfloat16)
    # 2b moving operands (plain matmuls)
    ablk = _blockdiag2(A.T).astype(bfloat16)
    nbblk = _blockdiag2(-Bm.T).astype(bfloat16)
    # W2 in DoubleRowSwInterleave raw layout
    w2a = np.zeros((128, KC, 128), np.float32)
    w2b = np.zeros((128, KC, 128), np.float32)
    for cc in range(KC):
        w2a[:, cc, :] = W2[cc * 128 : (cc + 1) * 128, 0:128].T
        w2b[0:64, cc, :] = W2[cc * 128 : (cc + 1) * 128, 128:192].T
    w2i = np.zeros((128, KC, 2, 128), np.float32)
    w2i[:, :, 0, :] = w2a[:, :, ::-1]   # raw even bytes: A_{127-k}
    w2i[:, :, 1, :] = w2b[:, :, ::-1]   # raw odd bytes:  B_{127-k}
    w2i = np.ascontiguousarray(
        w2i.transpose(0, 1, 3, 2).reshape(128, KC, 2, 128)).astype(float8_e4m3)
    onesb1 = np.zeros((128, 128 + 2 * DH), np.float32)
    onesb1[:, :128] = 1.0
    onesb1[:, 128 : 128 + DH] = b1 / 128.0
    onesb1[:, 128 + DH :] = b1 / 128.0
    onesb1 = onesb1.astype(bfloat16)

    in_maps = []
    for i in range(N_CORES):
        xs = x[i * B_LOC : (i + 1) * B_LOC]                 # [2,64,64,768]
        xsT = np.ascontiguousarray(xs.reshape(TOK, C).T)    # [768, TOK]
        # SwInterleave pairs: xT2[p, j, 2t+i] = xsT[(2j+i)*128+p, t]
        xT2_a = np.ascontiguousarray(
            xsT.reshape(3, 2, 128, TOK).transpose(2, 0, 3, 1).reshape(
                128, 3, 2 * TOK)).astype(float8_e4m3)
        in_maps.append(
            dict(xT2=xT2_a, w1p=w1p, w2i=w2i, ablk2a=ablk2a, bblk2a=bblk2a,
                 ablk=ablk, nbblk=nbblk, onesb1=onesb1)
        )
    return in_maps


def run(x, W1, b1, W2, b2, trace=False):
    nc = _get_nc()
    in_maps = make_in_maps(x, W1, b1, W2, b2)
    res = run_bass_kernel_spmd(nc, in_maps, core_ids=list(range(N_CORES)),
                               trace=trace)
    outs = []
    for i in range(N_CORES):
        o = np.asarray(res.results[i]["out"]).astype(np.float32)
        # o: [C, TOK] with token' = (b, w, h)
        o = o.reshape(C, B_LOC, W, H).transpose(1, 3, 2, 0)  # [b, h, w, c]
        outs.append(o)
    xs_full = np.concatenate(outs, axis=0)          # the adapter branch only
    full = x.astype(np.float32) + b2.astype(np.float32) + xs_full
    return full, res


def kernel(x, W1, b1, W2, b2):
    full, _ = run(np.asarray(x, dtype=np.float32), np.asarray(W1),
                  np.asarray(b1), np.asarray(W2), np.asarray(b2), trace=False)
    return full


# revision 28
# speedup vs baseline: 1.1979x; 1.1979x over previous
"""Trainium2 Bass kernel for the Adapter + FFT-low-pass nn.Module.

Math: the fft2 -> center-square mask -> ifft2 -> real -> abs block is a
linear operator separable over the two 64-sized spatial axes:
    Y = | A X A^T - B X B^T |   per (batch, channel) 64x64 image,
where C = IDFT @ diag(mask_unshifted) @ DFT (complex 64x64), A = Re C,
B = Im C.  Everything becomes TensorEngine matmuls.

Per core (2 of 16 batch images, 8192 tokens, pure data parallel):
  P1: h = gelu(x @ W1^T + b1)            tiles [tok(h-major), 192]
  P2: UA = (A over W) h ; UB = (B over W) h  (blockdiag stationary)
      scatter (b,h,w) -> (b,w,h) via internal-DRAM roundtrip
  P3: y = | (A over H) UA - (B over H) UB |  -> y_dr [128, 2, tok']
      (fp8 DoubleRow K-tile layout: block0 = d 0:128, block1 = d 128:192)
  P4: out[c, tok'] = W2 @ y  via fp8 DoubleRow matmuls, W2 stationary,
      K=192 in one matmul, N=512 tokens per matmul.
Software-pipelined: P3/P4 of image b-1 interleave with P1/P2 of image b
at tile-pair granularity to keep Tensor, Vector and Scalar all busy and
the PE HAM clock warm.  Skip connection + b2 are added host-side.

Output leaves in (c, b, w, h) order; host transposes back.
"""

import sys
import types

sys.path.insert(0, "/opt/trn_rl_repo")

import numpy as np

# ---------------------------------------------------------------------------
# optional NTFF profiling hook (used when trace=True; harmless otherwise)
if "antenv.axon_hooks" not in sys.modules:
    _hookmod = types.ModuleType("antenv.axon_hooks")
    _store = {}
    _hookmod.set_axon_ntff_profile_hook = lambda h: _store.__setitem__("v", h)
    _hookmod.get_axon_ntff_profile_hook = lambda: _store.get("v")
    sys.modules["antenv.axon_hooks"] = _hookmod
    try:
        from trn_agent_boot.trn_boot import _ntff_profile_via_ctypes

        _hookmod.set_axon_ntff_profile_hook(
            _ntff_profile_via_ctypes("/opt/axon/libaxon_pjrt.so")
        )
    except Exception:
        pass

import bass_rust
import concourse.bass as bass
import concourse.bacc as bacc
import concourse.mybir as mybir
import concourse.tile as tile
from concourse.bass_utils import run_bass_kernel_spmd
from concourse.tile_rust import add_dep_helper
from ml_dtypes import bfloat16, float8_e4m3

# ---------------------------------------------------------------------------
N_CORES = 8
B, H, W, C = 16, 64, 64, 768
DH = 192
B_LOC = B // N_CORES          # 2 batch images per core
TOK = B_LOC * H * W           # 8192 tokens per core
NT_B = H * W // 128           # 32 token tiles per batch image
NP_B = NT_B // 2              # 16 tile-pairs per image
KC = C // 128                 # 6 contraction chunks over channels
NG_B = H * W // 512           # 8 token groups (512) per image for stage3
F32 = mybir.dt.float32
BF16 = mybir.dt.bfloat16
FP8 = mybir.dt.float8e4
GELU = mybir.ActivationFunctionType.Gelu
ABS = mybir.ActivationFunctionType.Abs
COPY = mybir.ActivationFunctionType.Copy
DR = mybir.MatmulPerfMode.DoubleRow
DRSW = mybir.MatmulPerfMode.DoubleRowSwInterleave

DELAY_PAIRS = 4               # p3p4(b-1) trails p1p2(b) by this many pairs
DRAIN_PAT = "VSVSVS"          # p4 psum-drain engine per cc chunk


def _fft_mats():
    """A = Re(C), B = Im(C) with C = ifft(diag(m) fft(.)), N=64, RATE=.25."""
    n = 64
    line = int((n * n * 0.25) ** 0.5 // 2)
    m_shift = np.zeros(n, dtype=np.float64)
    m_shift[n // 2 - line : n // 2 + line] = 1.0
    m = np.fft.ifftshift(m_shift)
    F = np.fft.fft(np.eye(n), axis=0)
    Cm = (np.conj(F) / n) @ np.diag(m) @ F
    return np.real(Cm), np.imag(Cm)


def _blockdiag2(M):
    Z = np.zeros((128, 128), dtype=np.float64)
    Z[:64, :64] = M
    Z[64:, 64:] = M
    return Z


def build_bass():
    """Single-core Bass program, SPMD-replicated across the 8 cores."""
    nc = bacc.Bacc("TRN2", target_bir_lowering=False, debug=False,
                   num_devices=N_CORES)

    xT2 = nc.declare_dram_parameter("xT2", [128, 3, 2 * TOK], FP8,
                                    isOutput=False)
    w1p = nc.declare_dram_parameter("w1p", [128, 3, 2, DH], FP8,
                                    isOutput=False)
    w2i = nc.declare_dram_parameter("w2i", [128, KC, 2, 128], FP8,
                                    isOutput=False)
    ablk2a = nc.declare_dram_parameter("ablk2a", [128, 128], BF16,
                                       isOutput=False)
    bblk2a = nc.declare_dram_parameter("bblk2a", [128, 128], BF16,
                                       isOutput=False)
    ablk = nc.declare_dram_parameter("ablk", [128, 128], BF16, isOutput=False)
    nbblk = nc.declare_dram_parameter("nbblk", [128, 128], BF16, isOutput=False)
    onesb1 = nc.declare_dram_parameter("onesb1", [128, 128 + 2 * DH], BF16,
                                       isOutput=False)
    out = nc.declare_dram_parameter("out", [C, TOK], FP8, isOutput=True)

    # internal DRAM for the (b,h,w)->(b,w,h) scatter; [A-d | B-d] per token
    uab = nc.dram_tensor("uab", [B_LOC, H * W, 2 * DH], FP8)
    # scatter view: [b, h2, w, t, d] with token' = w*64 + (t*2 + h2)
    uab_sc = uab.rearrange("b (w t h2) d -> b h2 w t d", h2=2, t=NT_B)
    # 2b load view: [b, t4-group, p, i, d] with token' = t4*512 + i*128 + p
    uab_ld = uab.rearrange("b (t4 i p) d -> b t4 p i d", i=4, p=128)

    with tile.TileContext(nc) as tc:
        with (
            tc.tile_pool(name="const", bufs=1) as constp,
            tc.tile_pool(name="xt", bufs=4) as xtp,
            tc.tile_pool(name="h1", bufs=2) as h1p,
            tc.tile_pool(name="sa", bufs=2) as sap,
            tc.tile_pool(name="ub", bufs=5) as ubp,
            tc.tile_pool(name="yd", bufs=2) as ydp,
            tc.tile_pool(name="osb", bufs=2) as osbp,
            tc.tile_pool(name="ps1", bufs=2, space="PSUM") as ps1p,
            tc.tile_pool(name="ps2", bufs=2, space="PSUM") as ps2p,
            tc.tile_pool(name="ps3", bufs=2, space="PSUM") as ps3p,
            tc.tile_pool(name="ps4", bufs=2, space="PSUM") as ps4p,
        ):
            state = {}

            def load_xchunk(b, c):
                if ("xg", b, c) in state or c >= 8:
                    return
                t_ = xtp.tile([128, 3, 1024], FP8, tag="xg")
                nc.sync.dma_start(
                    t_[:], xT2[:, :, b * 8192 + c * 1024 :
                               b * 8192 + (c + 1) * 1024])
                state[("xg", b, c)] = t_

            # ---- first x chunks before the other constants: the first
            # stage1 matmul needs xg(0,0)+w1p+onesb1 only.
            load_xchunk(0, 0)
            w1p_sb = constp.tile([128, 3, 2, DH], FP8, tag="w1p")
            nc.sync.dma_start(w1p_sb[:], w1p[:])
            onesb1_sb = constp.tile([128, 128 + 2 * DH], BF16, tag="onesb1")
            nc.sync.dma_start(onesb1_sb[:], onesb1[:])
            load_xchunk(0, 1)
            ablk2a_sb = constp.tile([128, 128], BF16, tag="ablk2a")
            nc.gpsimd.dma_start(ablk2a_sb[:], ablk2a[:])
            bblk2a_sb = constp.tile([128, 128], BF16, tag="bblk2a")
            nc.gpsimd.dma_start(bblk2a_sb[:], bblk2a[:])
            ablk_sb = constp.tile([128, 128], BF16, tag="ablk")
            nc.gpsimd.dma_start(ablk_sb[:], ablk[:])
            nbblk_sb = constp.tile([128, 128], BF16, tag="nbblk")
            nc.gpsimd.dma_start(nbblk_sb[:], nbblk[:])
            w2i_sb = constp.tile([128, KC, 2, 128], FP8, tag="w2i")
            nc.gpsimd.dma_start(w2i_sb[:], w2i[:])
            ones_sb = onesb1_sb[:, 0:128]
            b1row2_sb = onesb1_sb[:, 128 : 128 + 2 * DH]

            # pre-zero PSUM banks used by p3: the batched abs reads a
            # never-written quadrant; keep it finite.  (ps1's zeroing is
            # deferred to after the head so it doesn't gate the first matmul.)
            for _ in range(2):
                z = ps3p.tile([128, 2, 2, 128], F32, tag="ps3")
                nc.vector.memset(z[:], 0.0)

            scat_dmas = [[], []]
            uab_fence = [None, None]

            def p12_pair(b, u):
                """stage1 + 2a for tiles 2u, 2u+1 of image b."""
                c = u // 2
                if u % 2 == 0:
                    load_xchunk(b, c)
                    load_xchunk(b, c + 1)
                    load_xchunk(b, c + 2)
                    if c >= 5 and b == 0:
                        load_xchunk(1, c - 5)
                if u == 0:
                    h1 = h1p.tile([128, NT_B, DH], FP8, tag="h1")
                    state[("h1", b)] = h1
                    sa = sap.tile([128, NT_B, 2 * DH], FP8, tag="sa")
                    state[("sa", b)] = sa
                h1 = state[("h1", b)]
                sa = state[("sa", b)]

                # --- stage1: bias first (sets has_written), then accumulate
                xg = state[("xg", b, u // 2)]
                hps = ps1p.tile([128, 2, DH], F32, tag="ps1")
                nc.tensor.matmul(hps[:], ones_sb, b1row2_sb,
                                 start=True, stop=False, skip_group_check=True)
                for i in range(2):
                    t = 2 * u + i
                    off = (t % 4) * 256
                    for j in range(3):
                        nc.tensor.matmul(
                            hps[:, i, :],
                            xg[:, j, off : off + 256].rearrange(
                                "p (i t) -> p i t", i=2),
                            w1p_sb[:, j, :, :], start=False,
                            stop=(i == 1 and j == 2),
                            skip_group_check=True, perf_mode=DRSW)
                nc.scalar.activation(h1[:, 2 * u : 2 * u + 2, :], hps[:], GELU)
                # --- 2a + sa copy
                for i in range(2):
                    t = 2 * u + i
                    aps = ps2p.tile([128, 2, DH], F32, tag="ps2")
                    nc.tensor.matmul(aps[:, 0, :], ablk2a_sb[:], h1[:, t, :],
                                     start=True, stop=True)
                    nc.tensor.matmul(aps[:, 1, :], bblk2a_sb[:],
                                     h1[:, t, :], start=True, stop=True)
                    if t % 8 >= 6:
                        nc.scalar.activation(sa[:, t, :], aps[:], COPY)
                    else:
                        nc.vector.tensor_copy(sa[:, t, :], aps[:])
                # --- scatter every 2 pairs (4 tiles); alternate DMA queues
                if u % 2 == 1:
                    t4 = u // 2
                    for h2 in range(2):
                        eng = nc.gpsimd if h2 == 0 else nc.sync
                        s = eng.dma_start(
                            uab_sc[b, h2, :, 4 * t4 : 4 * t4 + 4, :],
                            sa[h2 * 64 : (h2 + 1) * 64,
                               4 * t4 : 4 * t4 + 4, :])
                        scat_dmas[b].append(s.ins)
                if u == NP_B - 1:
                    fence = nc.sync.nop(hint=f"uab_fence_{b}", nofuse=True)
                    for s in scat_dmas[b]:
                        add_dep_helper(fence.ins, s,
                                       reason="uab fence on scatter writes")
                    uab_fence[b] = fence.ins
                    load_ub(b, 0)
                    load_ub(b, 1)
                    load_ub(b, 2)

            def load_ub(b, t4):
                if ("ubg", b, t4) in state or t4 >= NT_B // 4:
                    return
                ub = ubp.tile([128, 4, 2 * DH], FP8, tag="ub")
                ud = nc.gpsimd.dma_start(ub[:], uab_ld[b, t4, :, :, :])
                add_dep_helper(ud.ins, uab_fence[b],
                               reason="uab RAW: 2b read after 2a scatters")
                state[("ubg", b, t4)] = ub

            def p3_pair(b, u, pools):
                """2b for tiles 2u, 2u+1: y = |A.UA - B.UB| in DR layout."""
                if u == 0:
                    yd = ydp.tile([128, 2, H * W], FP8, tag="yd")
                    state[("yd", b)] = yd
                yd = state[("yd", b)]
                t4 = u // 2
                load_ub(b, t4)
                load_ub(b, t4 + 1)
                load_ub(b, t4 + 2)
                ub = state[("ubg", b, t4)]
                # psum layout [kt, i, tok]: kt-major so the batched abs AP
                # traversal matches yd's [kt, tok] order.
                pool, tg = pools[u % len(pools)]
                yps = pool.tile([128, 2, 2, 128], F32, tag=tg)
                for i in range(2):
                    j = (2 * u + i) % 4          # position within the ub group
                    nc.tensor.matmul(yps[:, 0, i, :], ub[:, j, 0:128],
                                     ablk_sb[:], start=True, stop=False,
                                     skip_group_check=True)
                    nc.tensor.matmul(yps[:, 0, i, :], ub[:, j, DH : DH + 128],
                                     nbblk_sb[:], start=False, stop=True,
                                     skip_group_check=True)
                    nc.tensor.matmul(yps[0:64, 1, i, :], ub[:, j, 128:DH],
                                     ablk_sb[:], start=True, stop=False,
                                     skip_group_check=True)
                    nc.tensor.matmul(yps[0:64, 1, i, :],
                                     ub[:, j, DH + 128 : 2 * DH],
                                     nbblk_sb[:], start=False, stop=True,
                                     skip_group_check=True)
                nc.scalar.activation(
                    yd[:, :, 2 * u * 128 : (2 * u + 2) * 128], yps[:], ABS)

            def p4_group(b, g, pools, pat=DRAIN_PAT):
                """stage3 for token group g: out[c, tok'] via fp8 DoubleRow."""
                if g == 0:
                    osb = osbp.tile([128, KC, H * W], FP8, tag="osb")
                    state[("osb", b)] = osb
                yd = state[("yd", b)]
                osb = state[("osb", b)]
                for cc in range(KC):
                    pool, tg = pools[cc % len(pools)]
                    ops = pool.tile([128, 512], F32, tag=tg)
                    drain_eng = pat[cc]
                    nc.tensor.matmul(
                        ops[:], w2i_sb[:, cc, :, :],
                        yd[:, :, g * 512 : (g + 1) * 512],
                        start=True, stop=True, perf_mode=DRSW)
                    if drain_eng == "V":
                        nc.vector.tensor_copy(
                            osb[:, cc, g * 512 : (g + 1) * 512], ops[:])
                    else:
                        nc.scalar.activation(
                            osb[:, cc, g * 512 : (g + 1) * 512], ops[:], COPY)
                if g == NG_B // 2 - 1 or g == NG_B - 1:
                    h0 = 0 if g < NG_B // 2 else H * W // 2
                    for cc in range(KC):
                        nc.sync.dma_start(
                            out[cc * 128 : (cc + 1) * 128,
                                b * H * W + h0 : b * H * W + h0 + H * W // 2],
                            osb[:, cc, h0 : h0 + H * W // 2])

            MID_P3 = [(ps3p, "ps3")]
            MID_P4 = [(ps4p, "ps4")]
            TAIL_P3 = [(ps3p, "ps3"), (ps1p, "ps1")]
            TAIL_P4 = [(ps4p, "ps4"), (ps2p, "ps2")]

            def p34_slot(b, v, tail):
                p3_pair(b, v, TAIL_P3 if tail else MID_P3)
                if v % 2 == 1:
                    p4_group(b, v // 2, TAIL_P4 if tail else MID_P4,
                             "VSVVSV" if tail else DRAIN_PAT)

            # ---- software-pipelined emission
            for u in range(NP_B):
                p12_pair(0, u)
            for _ in range(2):
                z = ps1p.tile([128, 2, 2, 128], F32, tag="ps1")
                nc.vector.memset(z[:], 0.0)
            for u in range(NP_B):
                p12_pair(1, u)
                v = u - DELAY_PAIRS
                if v >= 0:
                    p34_slot(0, v, tail=False)
            for v in range(NP_B - DELAY_PAIRS, NP_B):
                p34_slot(0, v, tail=True)
            for v in range(NP_B):
                p34_slot(1, v, tail=True)
    return nc


_NC_CACHE = {}


def _get_nc():
    if "nc" not in _NC_CACHE:
        nc = build_bass()
        nc.compile()
        _NC_CACHE["nc"] = nc
    return _NC_CACHE["nc"]


def make_in_maps(x, W1, b1, W2, b2):
    A, Bm = _fft_mats()
    # stage1 weights as fp8 DoubleRow pairs: w1p[p, j, i, d] = W1[d, (2j+i)*128+p]
    w1p = np.ascontiguousarray(
        W1.T.reshape(3, 2, 128, DH).transpose(2, 0, 1, 3)).astype(float8_e4m3)
    # 2a stationary filters, row-flipped to undo stage1's SwInterleave
    # token reversal (h1 partition p holds token 127-p).
    ablk2a = _blockdiag2(A.T)[::-1, :].astype(bfloat16)
    bblk2a = _blockdiag2(Bm.T)[::-1, :].astype(bfloat16)
    # 2b moving operands (plain matmuls)
    ablk = _blockdiag2(A.T).astype(bfloat16)
    nbblk = _blockdiag2(-Bm.T).astype(bfloat16)
    # W2 in DoubleRowSwInterleave raw layout
    w2a = np.zeros((128, KC, 128), np.float32)
    w2b = np.zeros((128, KC, 128), np.float32)
    for cc in range(KC):
        w2a[:, cc, :] = W2[cc * 128 : (cc + 1) * 128, 0:128].T
        w2b[0:64, cc, :] = W2[cc * 128 : (cc + 1) * 128, 128:192].T
    w2i = np.zeros((128, KC, 2, 128), np.float32)
    w2i[:, :, 0, :] = w2a[:, :, ::-1]   # raw even bytes: A_{127-k}
    w2i[:, :, 1, :] = w2b[:, :, ::-1]   # raw odd bytes:  B_{127-k}
    w2i = np.ascontiguousarray(
        w2i.transpose(0, 1, 3, 2).reshape(128, KC, 2, 128)).astype(float8_e4m3)
    onesb1 = np.zeros((128, 128 + 2 * DH), np.float32)
    onesb1[:, :128] = 1.0
    onesb1[:, 128 : 128 + DH] = b1 / 128.0
    onesb1[:, 128 + DH :] = b1 / 128.0
    onesb1 = onesb1.astype(bfloat16)

    in_maps = []
    for i in range(N_CORES):
        xs = x[i * B_LOC : (i + 1) * B_LOC]                 # [2,64,64,768]
        xsT = np.ascontiguousarray(xs.reshape(TOK, C).T)    # [768, TOK]
        # SwInterleave pairs: xT2[p, j, 2t+i] = xsT[(2j+i)*128+p, t]
        xT2_a = np.ascontiguousarray(
            xsT.reshape(3, 2, 128, TOK).transpose(2, 0, 3, 1).reshape(
                128, 3, 2 * TOK)).astype(float8_e4m3)
        in_maps.append(
            dict(xT2=xT2_a, w1p=w1p, w2i=w2i, ablk2a=ablk2a, bblk2a=bblk2a,
                 ablk=ablk, nbblk=nbblk, onesb1=onesb1)
        )
    return in_maps


def run(x, W1, b1, W2, b2, trace=False):
    nc = _get_nc()
    in_maps = make_in_maps(x, W1, b1, W2, b2)
    res = run_bass_kernel_spmd(nc, in_maps, core_ids=list(range(N_CORES)),
                               trace=trace)
    outs = []
    for i in range(N_CORES):
        o = np.asarray(res.results[i]["out"]).astype(np.float32)
        # o: [C, TOK] with token' = (b, w, h)
        o = o.reshape(C, B_LOC, W, H).transpose(1, 3, 2, 0)  # [b, h, w, c]
        outs.append(o)
    xs_full = np.concatenate(outs, axis=0)          # the adapter branch only
    full = x.astype(np.float32) + b2.astype(np.float32) + xs_full
    return full, res


def kernel(x, W1, b1, W2, b2):
    full, _ = run(np.asarray(x, dtype=np.float32), np.asarray(W1),
                  np.asarray(b1), np.asarray(W2), np.asarray(b2), trace=False)
    return full
